# revision 27
# baseline (speedup 1.0000x reference)
"""Trainium2 Bass kernel for a basis-customized linear layer.

Reference computation (B=1024, IN=OUT=512, EMB=64, KQ=64, NB=3, VOCAB=100):
    embs = concat(emb_author[idx_author], emb_citation[idx_citation])  # [B, 128]
    h    = tanh(embs @ W1.T + b1)                                      # [B, 64]
    coef = softmax(h @ W2.T)                                           # [B, 3]
    w    = (coef @ W3.T + b3).reshape(B, IN, OUT)
    out  = einsum('bi,bio->bo', x, w)                                  # [B, 512]

Rewrites:
  (1) w[b] = sum_j coef[b,j]*W3j + b3r and softmax coefs sum to 1, so
      out = sum_j coef[:,j] * (x @ (W3j + b3r)) -- three shared [512,512]
      matmuls instead of the 1GB per-sample weight.
  (2) Y_j[m] = x_m @ W_j accumulate over k into per-(m,j) PSUM banks; the
      per-sample combine sum_j coef[b,j]*Y_j[b,o] has b on PARTITIONS, so
      coef enters as a per-partition [128,1] scalar: one ACT scale-copy +
      two fused DVE scalar_tensor_tensor ((Y*c) + acc) ops per m-tile,
      pipelined behind the j-th matmul group.  Nothing gates the main
      matmuls except the x/W DMAs.
  (3) the embedding gather is a one-hot matmul (idx compared against an
      iota via a single fused K=2 matmul covering author+citation), fused
      with W1 through the host-precomputed per-vocab table
      G = emb @ W1half.T (param-only fold).
  (4) everything travels bf16 (half the HBM bytes, 2x PE rate); PSUM
      accumulation stays f32; the output returns bf16 and is widened to
      f32 on the host.
  (5) THREE input DMAs only -- each dma_start costs ~0.7us of sequencer
      issue time and each DMA dependency pays a ~2us completion-receipt
      before its semaphore fires, so small tensors are packed together:
      smx (gather tables on partitions 0..99, W2.T|b1, idx/iota rows at
      base partition 64 -- PE operands must start at partition 0/32/64)
      on the scalar queue, and wx = x.T | W-blocks on the sync queue in
      two chunks so chunk0 (x + k0 weights) releases the first matmuls
      with a single semaphore.
  (6) the HAM clock-gate runs the PE at half rate until it has seen a
      full activity window, so dummy matmuls on memset data keep the PE
      busy during the DMA wait; their target banks are re-cleared by the
      first real accumulation (start=True).

Sharding over 8 cores: batch 4-way x out-column 2-way.
"""

import numpy as np
import ml_dtypes

import concourse.bass as bass
import concourse.tile as tile
from concourse import bacc, mybir
from concourse.bass_utils import run_bass_kernel_spmd

# Problem dims (hardcoded per contract)
B, IN, OUT = 1024, 512, 512
EMB, KQ, NB, VOCAB = 64, 64, 3, 100
P_B, Q_O = 4, 2            # batch shards x out-col shards = 8 cores
BS = B // P_B              # 256 batch rows per core
OW = OUT // Q_O            # 256 out cols per core
KT = IN // 128             # 4 contraction tiles
MT = BS // 128             # 2 batch tiles per core

F32 = mybir.dt.float32
F32R = mybir.dt.float32r
BF16 = mybir.dt.bfloat16

# smx (all small operands, ONE dma -> one early semaphore), bf16 [128, 744]:
#   [0:64)    gat   rows 0:100   emb_author @ W1a.T
#   [64:128)  gct   rows 0:100   emb_citation @ W1c.T
#   [128:132) w2r   rows 0:64    W2.T | b1
#   [132:644) idx   rows 64-65   (author|citation, row65 = ones)
#   [644:744) bw    rows 64-65   (ones | -iota)   base partition 64 for PE
SMX = 132 + 2 * BS + VOCAB
# wx: x.T (k-packed) and the weight blocks share one tensor so chunk0
# (xt + k0 weights) releases the first matmuls with a single semaphore
XC = KT * BS
WXC = XC + KT * NB * OW

LAST_RESULT = None         # BassKernelResults of the most recent run (for test.py)

_NC_CACHE = None


def _ensure_ntff_hook_module():
    """bass_utils imports antenv.axon_hooks when BASS_TRACE is set; the module
    is absent on this image. Provide a no-op shim so tracing degrades
    gracefully instead of crashing."""
    import sys, types
    if "antenv.axon_hooks" in sys.modules:
        return
    try:
        import antenv
        import antenv.axon_hooks  # noqa: F401
    except ImportError:
        mod = types.ModuleType("antenv.axon_hooks")
        state = {"hook": None}
        mod.set_axon_ntff_profile_hook = lambda h: state.__setitem__("hook", h)
        mod.get_axon_ntff_profile_hook = lambda: state["hook"]
        sys.modules["antenv.axon_hooks"] = mod
        try:
            antenv.axon_hooks = mod
        except Exception:
            pass


def _build_nc():
    nc = bacc.Bacc("TRN2", target_bir_lowering=False, debug=False,
                   num_devices=P_B * Q_O)

    wx = nc.dram_tensor("wx", [128, WXC], BF16, kind="ExternalInput")
    smx = nc.dram_tensor("smx", [128, SMX], BF16, kind="ExternalInput")
    out = nc.dram_tensor("out", [128, MT * OW], BF16, kind="ExternalOutput")

    with tile.TileContext(nc) as tc:
        with (
            tc.tile_pool(name="consts", bufs=1) as consts,
            tc.tile_pool(name="ps", bufs=1, space="PSUM") as ps,
        ):
            # ---- 3 input DMAs: every dependency costs one ~2us receipt.
            # sync and scalar(Activation) are separate HWDGE rings, so the
            # two weight chunks' completion receipts process in parallel.
            smx_sb = consts.tile([128, SMX], BF16)
            nc.sync.dma_start(out=smx_sb, in_=smx[:, :])
            wx_sb = consts.tile([128, WXC], BF16)
            c0 = XC + NB * OW                          # xt + k0 weights
            nc.sync.dma_start(out=wx_sb[:, 0:c0], in_=wx[:, 0:c0])
            nc.scalar.dma_start(out=wx_sb[:, c0:], in_=wx[:, c0:])

            gat_sb = smx_sb[0:VOCAB, 0:KQ]
            gct_sb = smx_sb[0:VOCAB, KQ:2 * KQ]
            w2r_sb = smx_sb[0:KQ, 2 * KQ:2 * KQ + NB + 1]
            idx_sb = smx_sb[64:66, 132:132 + 2 * BS]   # author | citation
            bw_sb = smx_sb[64:66, 132 + 2 * BS:SMX]

            # ---- PE stream (in-order): interleave stage-A with main ----
            # warm-up: the HAM clock-gate needs ~3.4us of continuous PE
            # activity before matmuls run at full rate.  Burn the DMA-wait
            # window with dummy matmuls on memset-zero data; each one
            # start=True-clears the bank the real one-hot then overwrites.
            zz_sb = consts.tile([2, 512], BF16)
            nc.vector.memset(zz_sb, 0)
            oh_ps = ps.tile([VOCAB, 2 * BS], F32, tag="oh", bufs=1,
                            padded_shape=[128, 512])
            pre_ps = ps.tile([KQ, BS], F32, tag="pre", bufs=1,
                             padded_shape=[128, 512])
            for _ in range(18):
                nc.tensor.matmul(oh_ps[:, 0:128], lhsT=zz_sb[:, 0:VOCAB],
                                 rhs=zz_sb[:, 0:128], start=True, stop=True)

            y_ps = [[ps.tile([128, OW], F32, tag="y", bufs=2 * NB,
                             name=f"y{m}_{j}", padded_shape=[128, 512])
                     for j in range(NB)] for m in range(MT)]

            def mm(m, j, k):
                cs = XC + (k * NB + j) * OW
                nc.tensor.matmul(
                    y_ps[m][j],
                    lhsT=wx_sb[:, k * BS + m * 128:k * BS + (m + 1) * 128],
                    rhs=wx_sb[:, cs:cs + OW],
                    start=(k == 0), stop=(k == KT - 1),
                )

            # fused one-hot for both embeddings: psum[v, (a|c)b]
            nc.tensor.matmul(oh_ps, lhsT=bw_sb, rhs=idx_sb, start=True, stop=True)
            oh_sb = consts.tile([VOCAB, 2 * BS], BF16)
            nc.vector.tensor_scalar(
                out=oh_sb[:, 0:BS], in0=oh_ps[:, 0:BS], scalar1=0.0,
                scalar2=None, op0=mybir.AluOpType.is_equal,
            )
            nc.vector.tensor_scalar(
                out=oh_sb[:, BS:2 * BS], in0=oh_ps[:, BS:2 * BS], scalar1=0.0,
                scalar2=None, op0=mybir.AluOpType.is_equal,
            )

            # keep the PE busy until the first wx chunk's semaphore fires
            for _ in range(8):
                nc.tensor.matmul(pre_ps[:, 0:128], lhsT=zz_sb[:, 0:KQ],
                                 rhs=zz_sb[:, 0:128], start=True, stop=True)

            # main matmuls k-major: 6 MMs per k-tile chase the wx chunks;
            # stage-A MMs slot into the stream right when their deps land
            for m in range(MT):
                for j in range(NB):
                    mm(m, j, 0)

            # fused gather + W1: preact.T [KQ, BS]
            nc.tensor.matmul(pre_ps, lhsT=gat_sb, rhs=oh_sb[:, 0:BS],
                             start=True, stop=False)
            nc.tensor.matmul(pre_ps, lhsT=gct_sb, rhs=oh_sb[:, BS:2 * BS],
                             start=False, stop=True)

            for m in range(MT):
                for j in range(NB):
                    mm(m, j, 1)

            # b1 routed through ACT so Tanh's bias dep is same-engine
            b1_sb = consts.tile([KQ, 1], F32)
            nc.scalar.copy(out=b1_sb, in_=smx_sb[0:KQ, 2 * KQ + NB:2 * KQ + NB + 1])
            ht_sb = consts.tile([KQ, BS], BF16)
            nc.scalar.activation(
                out=ht_sb, in_=pre_ps, func=mybir.ActivationFunctionType.Tanh,
                bias=b1_sb, scale=1.0,
            )

            for m in range(MT):
                for j in range(NB):
                    mm(m, j, 2)

            # per-m logits -> unnormalized e -> coef = e / sum(e)
            # lg0 reuses the one-hot's PSUM bank, lg1 the preact's
            coef = []
            for m in range(MT):
                lg_ps = ps.tile([128, NB + 1], F32, tag="oh" if m == 0 else "pre",
                                bufs=1, padded_shape=[128, 512])
                nc.tensor.matmul(
                    lg_ps, lhsT=ht_sb[:, m * 128:(m + 1) * 128], rhs=w2r_sb,
                    start=True, stop=True,
                )
                e_sb = consts.tile([128, NB], F32, name=f"e{m}")
                nc.scalar.activation(
                    out=e_sb, in_=lg_ps[:, 0:NB],
                    func=mybir.ActivationFunctionType.Exp,
                )
                s_sb = consts.tile([128, 1], F32, name=f"s{m}")
                nc.vector.reduce_sum(out=s_sb, in_=e_sb, axis=mybir.AxisListType.X)
                r_sb = consts.tile([128, 1], F32, name=f"r{m}")
                nc.vector.reciprocal(out=r_sb, in_=s_sb)
                cf = consts.tile([128, NB], F32, name=f"coef{m}")
                nc.vector.tensor_scalar(
                    out=cf, in0=e_sb, scalar1=r_sb, scalar2=None,
                    op0=mybir.AluOpType.mult,
                )
                coef.append(cf)

            # last k-tile m-interleaved so both combines pipeline behind it
            for j in range(NB):
                for m in range(MT):
                    mm(m, j, 3)

            # ---- combine: out[m] = sum_j coef[:,j] * Y_j[m] ----
            a_sb = [consts.tile([128, OW], F32, name=f"a{m}") for m in range(MT)]
            b_sb = [consts.tile([128, OW], F32, name=f"b{m}") for m in range(MT)]
            out_sb = [consts.tile([128, OW], BF16, name=f"o{m}") for m in range(MT)]
            for m in range(MT):
                nc.scalar.activation(
                    out=a_sb[m], in_=y_ps[m][0],
                    func=mybir.ActivationFunctionType.Copy,
                    scale=coef[m][:, 0:1],
                )
            for m in range(MT):
                nc.vector.scalar_tensor_tensor(
                    out=b_sb[m], in0=y_ps[m][1], scalar=coef[m][:, 1:2],
                    in1=a_sb[m], op0=mybir.AluOpType.mult, op1=mybir.AluOpType.add,
                )
            for m in range(MT):
                nc.vector.scalar_tensor_tensor(
                    out=out_sb[m], in0=y_ps[m][2], scalar=coef[m][:, 2:3],
                    in1=b_sb[m], op0=mybir.AluOpType.mult, op1=mybir.AluOpType.add,
                )

            nc.scalar.dma_start(out=out[:, 0:OW], in_=out_sb[0])
            nc.sync.dma_start(out=out[:, OW:2 * OW], in_=out_sb[1])

    nc.compile()
    return nc


def _get_nc():
    global _NC_CACHE
    if _NC_CACHE is None:
        _NC_CACHE = _build_nc()
    return _NC_CACHE


def _make_in_maps(x, idx_author, idx_citation, emb_author, emb_citation,
                  W1, b1, W2, W3, b3):
    f = np.float32
    bf = ml_dtypes.bfloat16
    x = np.asarray(x, dtype=f)
    W3r = np.asarray(W3, dtype=f).reshape(IN, OUT, NB)
    b3r = np.asarray(b3, dtype=f).reshape(IN, OUT)
    W1 = np.asarray(W1, dtype=f)

    # param-only folds: per-vocab gather tables G = emb @ W1half.T [VOCAB, KQ]
    smx = np.zeros((128, SMX), bf)
    smx[:VOCAB, :KQ] = (np.asarray(emb_author, dtype=f) @ W1[:, :EMB].T).astype(bf)
    smx[:VOCAB, KQ:2 * KQ] = (np.asarray(emb_citation, dtype=f) @ W1[:, EMB:].T).astype(bf)
    smx[:KQ, 2 * KQ:2 * KQ + NB] = np.asarray(W2, dtype=f).T.astype(bf)
    smx[:KQ, 2 * KQ + NB] = np.asarray(b1, dtype=f).astype(bf)
    smx[65, 132:132 + 2 * BS] = 1
    smx[64, 132 + 2 * BS:SMX] = 1
    smx[65, 132 + 2 * BS:SMX] = (-np.arange(VOCAB, dtype=f)).astype(bf)

    # per out-shard weight blocks, bias folded in: [128, k, j, ow] bf16
    wc_blocks = []
    for oj in range(Q_O):
        cols = slice(oj * OW, (oj + 1) * OW)
        blk = W3r[:, cols, :] + b3r[:, cols, None]       # [IN, OW, NB]
        blk = blk.reshape(KT, 128, OW, NB).transpose(1, 0, 3, 2)
        wc_blocks.append(np.ascontiguousarray(
            blk.reshape(128, KT * NB * OW).astype(bf)))

    # x.T per batch shard, k packed: [128, KT*BS] bf16
    xt_shards = []
    for bi in range(P_B):
        xs = x[bi * BS:(bi + 1) * BS, :].T               # [IN, BS]
        xs = xs.reshape(KT, 128, BS).transpose(1, 0, 2)
        xt_shards.append(xs.reshape(128, KT * BS).astype(bf))

    ia = np.asarray(idx_author).astype(bf)
    ic = np.asarray(idx_citation).astype(bf)

    in_maps = []
    for c in range(P_B * Q_O):
        bi, oj = c // Q_O, c % Q_O  # 4 batch shards x 2 out shards
        rows = slice(bi * BS, (bi + 1) * BS)
        smxc = smx.copy()
        smxc[64, 132:132 + BS] = ia[rows]
        smxc[64, 132 + BS:132 + 2 * BS] = ic[rows]
        in_maps.append({
            "wx": np.ascontiguousarray(np.concatenate(
                [xt_shards[bi], wc_blocks[oj]], axis=1)),
            "smx": smxc,
        })
    return in_maps


def kernel(x, idx_author, idx_citation, emb_author, emb_citation,
           W1, b1, W2, W3, b3):
    global LAST_RESULT
    _ensure_ntff_hook_module()
    nc = _get_nc()
    in_maps = _make_in_maps(x, idx_author, idx_citation, emb_author,
                            emb_citation, W1, b1, W2, W3, b3)
    res = run_bass_kernel_spmd(nc, in_maps, core_ids=list(range(P_B * Q_O)))
    LAST_RESULT = res
    out = np.empty((B, OUT), dtype=np.float32)
    for c in range(P_B * Q_O):
        bi, oj = c // Q_O, c % Q_O
        blk = np.asarray(res.results[c]["out"], dtype=np.float32)
        blk = blk.reshape(128, MT, OW).transpose(1, 0, 2)
        out[bi * BS:(bi + 1) * BS, oj * OW:(oj + 1) * OW] = \
            blk.reshape(BS, OW)
    return out


# revision 28
# speedup vs baseline: 1.0159x; 1.0159x over previous
"""Trainium2 Bass kernel for a basis-customized linear layer.

Reference computation (B=1024, IN=OUT=512, EMB=64, KQ=64, NB=3, VOCAB=100):
    embs = concat(emb_author[idx_author], emb_citation[idx_citation])  # [B, 128]
    h    = tanh(embs @ W1.T + b1)                                      # [B, 64]
    coef = softmax(h @ W2.T)                                           # [B, 3]
    w    = (coef @ W3.T + b3).reshape(B, IN, OUT)
    out  = einsum('bi,bio->bo', x, w)                                  # [B, 512]

Rewrites:
  (1) w[b] = sum_j coef[b,j]*W3j + b3r and softmax coefs sum to 1, so
      out = sum_j coef[:,j] * (x @ (W3j + b3r)) -- three shared [512,512]
      matmuls instead of the 1GB per-sample weight.
  (2) Y_j[m] = x_m @ W_j accumulate over k into per-(m,j) PSUM banks; the
      per-sample combine sum_j coef[b,j]*Y_j[b,o] has b on PARTITIONS, so
      coef enters as a per-partition [128,1] scalar: one ACT scale-copy +
      two fused DVE scalar_tensor_tensor ((Y*c) + acc) ops per m-tile,
      pipelined behind the j-th matmul group.  Nothing gates the main
      matmuls except the x/W DMAs.
  (3) the embedding gather is a one-hot matmul (idx compared against an
      iota via a single fused K=2 matmul covering author+citation), fused
      with W1 through the host-precomputed per-vocab table
      G = emb @ W1half.T (param-only fold).
  (4) everything travels bf16 (half the HBM bytes, 2x PE rate); PSUM
      accumulation stays f32; the output returns bf16 and is widened to
      f32 on the host.
  (5) THREE input DMAs only -- each dma_start costs ~0.7us of sequencer
      issue time and each DMA dependency pays a ~2us completion-receipt
      before its semaphore fires, so small tensors are packed together:
      smx (gather tables on partitions 0..99, W2.T|b1, idx/iota rows at
      base partition 64 -- PE operands must start at partition 0/32/64)
      on the scalar queue, and wx = x.T | W-blocks on the sync queue in
      two chunks so chunk0 (x + k0 weights) releases the first matmuls
      with a single semaphore.
  (6) the HAM clock-gate runs the PE at half rate until it has seen a
      full activity window, so dummy matmuls on memset data keep the PE
      busy during the DMA wait; their target banks are re-cleared by the
      first real accumulation (start=True).

Sharding over 8 cores: batch 4-way x out-column 2-way.
"""

import numpy as np
import ml_dtypes

import concourse.bass as bass
import concourse.tile as tile
from concourse import bacc, mybir
from concourse.bass_utils import run_bass_kernel_spmd

# Problem dims (hardcoded per contract)
B, IN, OUT = 1024, 512, 512
EMB, KQ, NB, VOCAB = 64, 64, 3, 100
P_B, Q_O = 4, 2            # batch shards x out-col shards = 8 cores
BS = B // P_B              # 256 batch rows per core
OW = OUT // Q_O            # 256 out cols per core
KT = IN // 128             # 4 contraction tiles
MT = BS // 128             # 2 batch tiles per core

F32 = mybir.dt.float32
F32R = mybir.dt.float32r
BF16 = mybir.dt.bfloat16

# smx (all small operands, ONE dma -> one early semaphore), bf16 [128, 744]:
#   [0:64)    gat   rows 0:100   emb_author @ W1a.T
#   [64:128)  gct   rows 0:100   emb_citation @ W1c.T
#   [128:132) w2r   rows 0:64    W2.T | b1
#   [132:644) idx   rows 64-65   (author|citation, row65 = ones)
#   [644:744) bw    rows 64-65   (ones | -iota)   base partition 64 for PE
SMX = 132 + 2 * BS + VOCAB
# wx: x.T (k-packed) and the weight blocks share one tensor so chunk0
# (xt + k0 weights) releases the first matmuls with a single semaphore
XC = KT * BS
WXC = XC + KT * NB * OW

LAST_RESULT = None         # BassKernelResults of the most recent run (for test.py)

_NC_CACHE = None


def _ensure_ntff_hook_module():
    """bass_utils imports antenv.axon_hooks when BASS_TRACE is set; the module
    is absent on this image. Provide a no-op shim so tracing degrades
    gracefully instead of crashing."""
    import sys, types
    if "antenv.axon_hooks" in sys.modules:
        return
    try:
        import antenv
        import antenv.axon_hooks  # noqa: F401
    except ImportError:
        mod = types.ModuleType("antenv.axon_hooks")
        state = {"hook": None}
        mod.set_axon_ntff_profile_hook = lambda h: state.__setitem__("hook", h)
        mod.get_axon_ntff_profile_hook = lambda: state["hook"]
        sys.modules["antenv.axon_hooks"] = mod
        try:
            antenv.axon_hooks = mod
        except Exception:
            pass


def _build_nc():
    nc = bacc.Bacc("TRN2", target_bir_lowering=False, debug=False,
                   num_devices=P_B * Q_O)

    wx = nc.dram_tensor("wx", [128, WXC], BF16, kind="ExternalInput")
    smx = nc.dram_tensor("smx", [128, SMX], BF16, kind="ExternalInput")
    out = nc.dram_tensor("out", [128, MT * OW], BF16, kind="ExternalOutput")

    with tile.TileContext(nc) as tc:
        with (
            tc.tile_pool(name="consts", bufs=1) as consts,
            tc.tile_pool(name="ps", bufs=1, space="PSUM") as ps,
        ):
            # ---- 3 input DMAs: every dependency costs one ~2us receipt.
            # sync and scalar(Activation) are separate HWDGE rings, so the
            # two weight chunks' completion receipts process in parallel.
            smx_sb = consts.tile([128, SMX], BF16)
            nc.sync.dma_start(out=smx_sb, in_=smx[:, :])
            wx_sb = consts.tile([128, WXC], BF16)
            c0 = XC + NB * OW                          # xt + k0 weights
            nc.sync.dma_start(out=wx_sb[:, 0:c0], in_=wx[:, 0:c0])
            nc.sync.dma_start(out=wx_sb[:, c0:], in_=wx[:, c0:])

            gat_sb = smx_sb[0:VOCAB, 0:KQ]
            gct_sb = smx_sb[0:VOCAB, KQ:2 * KQ]
            w2r_sb = smx_sb[0:KQ, 2 * KQ:2 * KQ + NB + 1]
            idx_sb = smx_sb[64:66, 132:132 + 2 * BS]   # author | citation
            bw_sb = smx_sb[64:66, 132 + 2 * BS:SMX]

            # ---- PE stream (in-order): interleave stage-A with main ----
            # warm-up: the HAM clock-gate needs ~3.4us of continuous PE
            # activity before matmuls run at full rate.  Burn the DMA-wait
            # window with dummy matmuls on memset-zero data; each one
            # start=True-clears the bank the real one-hot then overwrites.
            zz_sb = consts.tile([2, 512], BF16)
            nc.vector.memset(zz_sb, 0)
            oh_ps = ps.tile([VOCAB, 2 * BS], F32, tag="oh", bufs=1,
                            padded_shape=[128, 512])
            pre_ps = ps.tile([KQ, BS], F32, tag="pre", bufs=1,
                             padded_shape=[128, 512])
            for _ in range(18):
                nc.tensor.matmul(oh_ps[:, 0:128], lhsT=zz_sb[:, 0:VOCAB],
                                 rhs=zz_sb[:, 0:128], start=True, stop=True)

            y_ps = [[ps.tile([128, OW], F32, tag="y", bufs=2 * NB,
                             name=f"y{m}_{j}", padded_shape=[128, 512])
                     for j in range(NB)] for m in range(MT)]

            def mm(m, j, k):
                cs = XC + (k * NB + j) * OW
                nc.tensor.matmul(
                    y_ps[m][j],
                    lhsT=wx_sb[:, k * BS + m * 128:k * BS + (m + 1) * 128],
                    rhs=wx_sb[:, cs:cs + OW],
                    start=(k == 0), stop=(k == KT - 1),
                )

            # fused one-hot for both embeddings: psum[v, (a|c)b]
            nc.tensor.matmul(oh_ps, lhsT=bw_sb, rhs=idx_sb, start=True, stop=True)
            oh_sb = consts.tile([VOCAB, 2 * BS], BF16)
            nc.vector.tensor_scalar(
                out=oh_sb[:, 0:BS], in0=oh_ps[:, 0:BS], scalar1=0.0,
                scalar2=None, op0=mybir.AluOpType.is_equal,
            )
            nc.vector.tensor_scalar(
                out=oh_sb[:, BS:2 * BS], in0=oh_ps[:, BS:2 * BS], scalar1=0.0,
                scalar2=None, op0=mybir.AluOpType.is_equal,
            )

            # keep the PE busy until the first wx chunk's semaphore fires
            for _ in range(8):
                nc.tensor.matmul(pre_ps[:, 0:128], lhsT=zz_sb[:, 0:KQ],
                                 rhs=zz_sb[:, 0:128], start=True, stop=True)

            # main matmuls k-major: 6 MMs per k-tile chase the wx chunks;
            # stage-A MMs slot into the stream right when their deps land
            for m in range(MT):
                for j in range(NB):
                    mm(m, j, 0)

            # fused gather + W1: preact.T [KQ, BS]
            nc.tensor.matmul(pre_ps, lhsT=gat_sb, rhs=oh_sb[:, 0:BS],
                             start=True, stop=False)
            nc.tensor.matmul(pre_ps, lhsT=gct_sb, rhs=oh_sb[:, BS:2 * BS],
                             start=False, stop=True)

            for m in range(MT):
                for j in range(NB):
                    mm(m, j, 1)

            # b1 routed through ACT so Tanh's bias dep is same-engine
            b1_sb = consts.tile([KQ, 1], F32)
            nc.scalar.copy(out=b1_sb, in_=smx_sb[0:KQ, 2 * KQ + NB:2 * KQ + NB + 1])
            ht_sb = consts.tile([KQ, BS], BF16)
            nc.scalar.activation(
                out=ht_sb, in_=pre_ps, func=mybir.ActivationFunctionType.Tanh,
                bias=b1_sb, scale=1.0,
            )

            for m in range(MT):
                for j in range(NB):
                    mm(m, j, 2)

            # per-m logits -> unnormalized e -> coef = e / sum(e)
            # lg0 reuses the one-hot's PSUM bank, lg1 the preact's
            coef = []
            for m in range(MT):
                lg_ps = ps.tile([128, NB + 1], F32, tag="oh" if m == 0 else "pre",
                                bufs=1, padded_shape=[128, 512])
                nc.tensor.matmul(
                    lg_ps, lhsT=ht_sb[:, m * 128:(m + 1) * 128], rhs=w2r_sb,
                    start=True, stop=True,
                )
                e_sb = consts.tile([128, NB], F32, name=f"e{m}")
                nc.scalar.activation(
                    out=e_sb, in_=lg_ps[:, 0:NB],
                    func=mybir.ActivationFunctionType.Exp,
                )
                s_sb = consts.tile([128, 1], F32, name=f"s{m}")
                nc.vector.reduce_sum(out=s_sb, in_=e_sb, axis=mybir.AxisListType.X)
                r_sb = consts.tile([128, 1], F32, name=f"r{m}")
                nc.vector.reciprocal(out=r_sb, in_=s_sb)
                cf = consts.tile([128, NB], F32, name=f"coef{m}")
                nc.vector.tensor_scalar(
                    out=cf, in0=e_sb, scalar1=r_sb, scalar2=None,
                    op0=mybir.AluOpType.mult,
                )
                coef.append(cf)

            # last k-tile m-interleaved so both combines pipeline behind it
            for j in range(NB):
                for m in range(MT):
                    mm(m, j, 3)

            # ---- combine: out[m] = sum_j coef[:,j] * Y_j[m] ----
            a_sb = [consts.tile([128, OW], F32, name=f"a{m}") for m in range(MT)]
            b_sb = [consts.tile([128, OW], F32, name=f"b{m}") for m in range(MT)]
            out_sb = [consts.tile([128, OW], BF16, name=f"o{m}") for m in range(MT)]
            for m in range(MT):
                nc.scalar.activation(
                    out=a_sb[m], in_=y_ps[m][0],
                    func=mybir.ActivationFunctionType.Copy,
                    scale=coef[m][:, 0:1],
                )
            for m in range(MT):
                nc.vector.scalar_tensor_tensor(
                    out=b_sb[m], in0=y_ps[m][1], scalar=coef[m][:, 1:2],
                    in1=a_sb[m], op0=mybir.AluOpType.mult, op1=mybir.AluOpType.add,
                )
            for m in range(MT):
                nc.vector.scalar_tensor_tensor(
                    out=out_sb[m], in0=y_ps[m][2], scalar=coef[m][:, 2:3],
                    in1=b_sb[m], op0=mybir.AluOpType.mult, op1=mybir.AluOpType.add,
                )

            nc.scalar.dma_start(out=out[:, 0:OW], in_=out_sb[0])
            nc.sync.dma_start(out=out[:, OW:2 * OW], in_=out_sb[1])

    nc.compile()
    return nc


def _get_nc():
    global _NC_CACHE
    if _NC_CACHE is None:
        _NC_CACHE = _build_nc()
    return _NC_CACHE


def _make_in_maps(x, idx_author, idx_citation, emb_author, emb_citation,
                  W1, b1, W2, W3, b3):
    f = np.float32
    bf = ml_dtypes.bfloat16
    x = np.asarray(x, dtype=f)
    W3r = np.asarray(W3, dtype=f).reshape(IN, OUT, NB)
    b3r = np.asarray(b3, dtype=f).reshape(IN, OUT)
    W1 = np.asarray(W1, dtype=f)

    # param-only folds: per-vocab gather tables G = emb @ W1half.T [VOCAB, KQ]
    smx = np.zeros((128, SMX), bf)
    smx[:VOCAB, :KQ] = (np.asarray(emb_author, dtype=f) @ W1[:, :EMB].T).astype(bf)
    smx[:VOCAB, KQ:2 * KQ] = (np.asarray(emb_citation, dtype=f) @ W1[:, EMB:].T).astype(bf)
    smx[:KQ, 2 * KQ:2 * KQ + NB] = np.asarray(W2, dtype=f).T.astype(bf)
    smx[:KQ, 2 * KQ + NB] = np.asarray(b1, dtype=f).astype(bf)
    smx[65, 132:132 + 2 * BS] = 1
    smx[64, 132 + 2 * BS:SMX] = 1
    smx[65, 132 + 2 * BS:SMX] = (-np.arange(VOCAB, dtype=f)).astype(bf)

    # per out-shard weight blocks, bias folded in: [128, k, j, ow] bf16
    wc_blocks = []
    for oj in range(Q_O):
        cols = slice(oj * OW, (oj + 1) * OW)
        blk = W3r[:, cols, :] + b3r[:, cols, None]       # [IN, OW, NB]
        blk = blk.reshape(KT, 128, OW, NB).transpose(1, 0, 3, 2)
        wc_blocks.append(np.ascontiguousarray(
            blk.reshape(128, KT * NB * OW).astype(bf)))

    # x.T per batch shard, k packed: [128, KT*BS] bf16
    xt_shards = []
    for bi in range(P_B):
        xs = x[bi * BS:(bi + 1) * BS, :].T               # [IN, BS]
        xs = xs.reshape(KT, 128, BS).transpose(1, 0, 2)
        xt_shards.append(xs.reshape(128, KT * BS).astype(bf))

    ia = np.asarray(idx_author).astype(bf)
    ic = np.asarray(idx_citation).astype(bf)

    in_maps = []
    for c in range(P_B * Q_O):
        bi, oj = c // Q_O, c % Q_O  # 4 batch shards x 2 out shards
        rows = slice(bi * BS, (bi + 1) * BS)
        smxc = smx.copy()
        smxc[64, 132:132 + BS] = ia[rows]
        smxc[64, 132 + BS:132 + 2 * BS] = ic[rows]
        in_maps.append({
            "wx": np.ascontiguousarray(np.concatenate(
                [xt_shards[bi], wc_blocks[oj]], axis=1)),
            "smx": smxc,
        })
    return in_maps


def kernel(x, idx_author, idx_citation, emb_author, emb_citation,
           W1, b1, W2, W3, b3):
    global LAST_RESULT
    _ensure_ntff_hook_module()
    nc = _get_nc()
    in_maps = _make_in_maps(x, idx_author, idx_citation, emb_author,
                            emb_citation, W1, b1, W2, W3, b3)
    res = run_bass_kernel_spmd(nc, in_maps, core_ids=list(range(P_B * Q_O)))
    LAST_RESULT = res
    out = np.empty((B, OUT), dtype=np.float32)
    for c in range(P_B * Q_O):
        bi, oj = c // Q_O, c % Q_O
        blk = np.asarray(res.results[c]["out"], dtype=np.float32)
        blk = blk.reshape(128, MT, OW).transpose(1, 0, 2)
        out[bi * BS:(bi + 1) * BS, oj * OW:(oj + 1) * OW] = \
            blk.reshape(BS, OW)
    return out
